# revision 5
# baseline (speedup 1.0000x reference)
"""Trainium2 Bass kernel for pointer-generator attention (nn_Attention_35296041239157).

Math (per batch row b):
    s_t      = concat(h_dec, c_dec)                      # (D,)
    dec_feat = s_t @ Ws_w + Ws_b                         # (D,)
    att      = enc @ Wh + dec_feat + cov[:, None] * wc   # (L, D)
    score    = tanh(att) @ v                             # (L,)
    w        = exp(score) * mask                         # (L,)   (no max-sub: |score| <~ 16)
    attn     = w / sum(w)
    context  = attn @ enc                                # (D,)
    covn     = cov + attn

Distribution: pure data-parallel over batch B=16 -> 2 batches on each of the
8 NeuronCores; the small projection weights are replicated. No collectives.

Device layout: everything runs in the transposed [D, L] orientation so the
tanh-then-dot-v reduction happens on the TensorEngine (contraction over
partitions). enc tiles are transposed on-chip with PE transpose-mode.
dec_feat and the coverage rank-1 term are folded into the big matmul as a
K=2 accumulation (lhsT rows = [wc_chunk, dec_chunk], rhs rows = [cov_row,
ones]). The context matvec uses the natural-layout enc tiles (contraction
over L) with unnormalized weights, normalized once at the end.

Matmuls run in float32r (4-byte fp32 storage, full-rate PE streaming,
~1e-4 relative rounding). The BIR verifier requires every matmul operand
to carry f32r provenance, so matmul-feeding DRAM tensors are declared
float32r and on-chip producers (DVE copies, ACT activations) write f32r.
"""

import numpy as np

import concourse.bacc as bacc
import concourse.mybir as mybir
from concourse.tile import TileContext
from concourse.bass_utils import run_bass_kernel_spmd

B, L, D = 16, 4096, 1024
NCORES = 8
BL = B // NCORES          # 2 batches per core
P = 128
LT = 512                  # L supertile
NST = L // LT             # 8
NJ = LT // P              # 4 natural subtiles per supertile
KC = D // P               # 8 contraction / output chunks
NH = D // 512             # 2 halves of D for N<=512 matmuls

f32 = mybir.dt.float32
f32r = mybir.dt.float32r
FT = mybir.ActivationFunctionType
AX = mybir.AxisListType
ALU = mybir.AluOpType

ENC_NAT_BUFS = 10
ENCT_BUFS = 16


def build():
    nc = bacc.Bacc("TRN2", target_bir_lowering=False, debug=False)

    enc = nc.dram_tensor("enc", [BL, L, D], f32r, kind="ExternalInput")
    sT = nc.dram_tensor("sT", [D, BL], f32r, kind="ExternalInput")
    auxr_d = nc.dram_tensor("auxr", [BL, 2, L], f32r, kind="ExternalInput")
    cov = nc.dram_tensor("cov", [BL, L], f32r, kind="ExternalInput")
    mask = nc.dram_tensor("mask", [BL, L], f32, kind="ExternalInput")
    Wh = nc.dram_tensor("Wh", [D, D], f32r, kind="ExternalInput")
    Ws = nc.dram_tensor("Ws", [D, D], f32r, kind="ExternalInput")
    Wsb = nc.dram_tensor("Wsb", [1, D], f32, kind="ExternalInput")
    vcols = nc.dram_tensor("vcols", [P, KC], f32r, kind="ExternalInput")
    wc = nc.dram_tensor("wc", [1, D], f32r, kind="ExternalInput")
    ident_d = nc.dram_tensor("ident", [P, P], f32r, kind="ExternalInput")

    attn_d = nc.dram_tensor("attn", [BL, L], f32, kind="ExternalOutput")
    covn_d = nc.dram_tensor("covn", [BL, L], f32, kind="ExternalOutput")
    ctx_d = nc.dram_tensor("ctx", [BL, D], f32, kind="ExternalOutput")

    with TileContext(nc) as tc:
        with tc.tile_pool(name="const", bufs=1) as const:
            wh_sb = const.tile([P, KC, D], f32r)
            vc_sb = const.tile([P, KC], f32r)
            sT_sb = const.tile([P, KC, BL], f32r)
            wsb2 = const.tile([BL, D], f32)
            ident = const.tile([P, P], f32r)
            aux_l0 = const.tile([BL, D], f32r)  # row0 = wc, row1 = dec_feat[0]
            aux_l1 = const.tile([BL, D], f32r)
            dec_nat = const.tile([BL, D], f32r)
            wboth = const.tile([BL, L], f32r)   # w rows; becomes attn in place
            ctx0 = const.tile([1, D], f32)
            ctx1 = const.tile([1, D], f32)
            ctxb = const.tile([BL, D], f32)
            denom = const.tile([BL, 1], f32)
            recip = const.tile([BL, 1], f32)

            for k in range(KC):
                nc.sync.dma_start(out=wh_sb[:, k, :], in_=Wh[k * P:(k + 1) * P, :])
                nc.sync.dma_start(out=sT_sb[:, k, :], in_=sT[k * P:(k + 1) * P, :])
            nc.sync.dma_start(out=vc_sb[:], in_=vcols[:, :])
            nc.sync.dma_start(out=ident[:], in_=ident_d[:, :])
            for b in range(BL):
                nc.sync.dma_start(out=wsb2[b:b + 1, :], in_=Wsb[0:1, :])
            nc.sync.dma_start(out=aux_l0[0:1, :], in_=wc[0:1, :])
            nc.sync.dma_start(out=aux_l1[0:1, :], in_=wc[0:1, :])

            # dec_feat = s_t @ Ws + Ws_b   (natural rows [BL, D])
            with tc.tile_pool(name="wsp", bufs=1) as wsp, \
                 tc.tile_pool(name="ps0", bufs=2, space="PSUM") as ps0:
                ws_sb = wsp.tile([P, KC, D], f32r)
                for k in range(KC):
                    nc.sync.dma_start(out=ws_sb[:, k, :], in_=Ws[k * P:(k + 1) * P, :])
                for n in range(NH):
                    dp = ps0.tile([BL, 512], f32, name="dec_ps")
                    for k in range(KC):
                        nc.tensor.matmul(dp[:], sT_sb[:, k, :],
                                         ws_sb[:, k, n * 512:(n + 1) * 512],
                                         start=(k == 0), stop=(k == KC - 1))
                    nc.vector.tensor_add(dec_nat[:, n * 512:(n + 1) * 512], dp[:],
                                         wsb2[:, n * 512:(n + 1) * 512])

            # aux lhsT row 1 <- dec_feat rows (cross-partition, via DMA)
            nc.sync.dma_start(out=aux_l0[1:2, :], in_=dec_nat[0:1, :])
            nc.sync.dma_start(out=aux_l1[1:2, :], in_=dec_nat[1:2, :])

            with tc.tile_pool(name="work", bufs=3) as work, \
                 tc.tile_pool(name="psum", bufs=2, space="PSUM") as psum:

                for st in range(NST):
                    L0 = st * LT
                    # --- load natural enc tiles -------------------------------
                    nat = {}
                    for b in range(BL):
                        for j in range(NJ):
                            t = work.tile([P, D], f32r, name="enc_nat",
                                          tag="enc_nat", bufs=ENC_NAT_BUFS)
                            nc.sync.dma_start(
                                out=t[:], in_=enc[b, L0 + j * P:L0 + (j + 1) * P, :])
                            nat[b, j] = t

                    # --- transpose to [D, L] ----------------------------------
                    encT = {}
                    for b in range(BL):
                        for k in range(KC):
                            pt = psum.tile([P, LT], f32r, name="encT_ps", tag="mmps",
                                           bufs=4)
                            for j in range(NJ):
                                nc.tensor.transpose(
                                    pt[:, j * P:(j + 1) * P],
                                    nat[b, j][:, k * P:(k + 1) * P],
                                    ident[:])
                            et = work.tile([P, LT], f32r, name="encT", tag="encT",
                                           bufs=ENCT_BUFS)
                            nc.vector.tensor_copy(et[:], pt[:])
                            encT[b, k] = et

                    # --- attention scores -------------------------------------
                    for b in range(BL):
                        aux_l = aux_l0 if b == 0 else aux_l1
                        # aux rhs rows: [cov_row ; ones] straight from DRAM
                        auxr = work.tile([2, LT], f32r, name="auxr", tag="auxr",
                                         bufs=4)
                        nc.sync.dma_start(out=auxr[:], in_=auxr_d[b, :, L0:L0 + LT])

                        sc_ps = psum.tile([1, LT], f32, name="score_ps", tag="score",
                                          bufs=2)
                        pend = None
                        for m in range(KC):
                            ap_ = psum.tile([P, LT], f32, name="att_ps", tag="mmps",
                                            bufs=4)
                            nc.tensor.matmul(ap_[:], aux_l[:, m * P:(m + 1) * P],
                                             auxr[:], start=True, stop=False)
                            for k in range(KC):
                                nc.tensor.matmul(ap_[:],
                                                 wh_sb[:, k, m * P:(m + 1) * P],
                                                 encT[b, k][:],
                                                 start=False, stop=(k == KC - 1))
                            th = work.tile([P, LT], f32r, name="tanhT", tag="tanhT",
                                           bufs=3)
                            nc.scalar.activation(th[:], ap_[:], FT.Tanh)
                            if pend is not None:
                                nc.tensor.matmul(sc_ps[:],
                                                 vc_sb[:, pend[1]:pend[1] + 1],
                                                 pend[0][:],
                                                 start=(pend[1] == 0), stop=False)
                            pend = (th, m)
                        nc.tensor.matmul(sc_ps[:], vc_sb[:, pend[1]:pend[1] + 1],
                                         pend[0][:], start=False, stop=True)

                        # w = exp(score) * mask  -> row b of wboth
                        msk = work.tile([1, LT], f32, name="msk", tag="msk", bufs=4)
                        nc.sync.dma_start(out=msk[:], in_=mask[b:b + 1, L0:L0 + LT])
                        if b == 0:
                            wrow = wboth[0:1, L0:L0 + LT]
                            nc.scalar.activation(wrow, sc_ps[:], FT.Exp)
                            nc.vector.tensor_mul(wrow, wrow, msk[:])
                        else:
                            wtmp = work.tile([1, LT], f32r, name="wtmp", tag="wtmp",
                                             bufs=2)
                            nc.scalar.activation(wtmp[:], sc_ps[:], FT.Exp)
                            nc.vector.tensor_mul(wtmp[:], wtmp[:], msk[:])
                            nc.sync.dma_start(out=wboth[1:2, L0:L0 + LT], in_=wtmp[:])

                    # --- w columns (both batches) -----------------------------
                    wc_ps = psum.tile([P, BL * NJ], f32r, name="wcol_ps", tag="mmps",
                                      bufs=4)
                    for j in range(NJ):
                        nc.tensor.transpose(wc_ps[:, BL * j:BL * (j + 1)],
                                            wboth[:, L0 + j * P:L0 + (j + 1) * P],
                                            ident[0:BL, 0:BL])
                    wcol = work.tile([P, BL * NJ], f32r, name="wcol", tag="wcol",
                                     bufs=2)
                    nc.scalar.copy(wcol[:], wc_ps[:])

                    # --- context accumulation ---------------------------------
                    for b in range(BL):
                        ctx_acc = ctx0 if b == 0 else ctx1
                        for n in range(NH):
                            cp = psum.tile([1, 512], f32, name="ctx_ps", tag="ctxp",
                                           bufs=2)
                            for j in range(NJ):
                                nc.tensor.matmul(
                                    cp[:], wcol[:, BL * j + b:BL * j + b + 1],
                                    nat[b, j][:, n * 512:(n + 1) * 512],
                                    start=(j == 0), stop=(j == NJ - 1))
                            dst = ctx_acc[0:1, n * 512:(n + 1) * 512]
                            if st == 0:
                                nc.vector.tensor_copy(dst, cp[:])
                            else:
                                nc.vector.tensor_add(dst, dst, cp[:])

                # --- epilogue: softmax, coverage, context ---------------------
                covr = work.tile([BL, L], f32r, name="covr", tag="covr", bufs=1)
                nc.sync.dma_start(out=covr[:], in_=cov[:, :])

                nc.vector.tensor_reduce(out=denom[:], in_=wboth[:], axis=AX.X,
                                        op=ALU.add)
                nc.vector.reciprocal(recip[:], denom[:])
                nc.vector.tensor_scalar(wboth[:], wboth[:], recip[:], None, ALU.mult)
                nc.vector.tensor_add(covr[:], covr[:], wboth[:])

                nc.vector.tensor_copy(ctxb[0:1, :], ctx0[:])
                nc.sync.dma_start(out=ctxb[1:2, :], in_=ctx1[0:1, :])
                nc.vector.tensor_scalar(ctxb[:], ctxb[:], recip[:], None, ALU.mult)

                nc.sync.dma_start(out=attn_d[:, :], in_=wboth[:].bitcast(f32))
                nc.sync.dma_start(out=covn_d[:, :], in_=covr[:].bitcast(f32))
                nc.sync.dma_start(out=ctx_d[:, :], in_=ctxb[:])

    nc.compile()
    return nc


_NC_CACHE = None


def _get_nc():
    global _NC_CACHE
    if _NC_CACHE is None:
        _NC_CACHE = build()
    return _NC_CACHE


def kernel(**inputs):
    enc = np.ascontiguousarray(np.asarray(inputs["encoder_output"], np.float32))
    h = np.asarray(inputs["h_dec"], np.float32)[0]     # (B, 512)
    c = np.asarray(inputs["c_dec"], np.float32)[0]     # (B, 512)
    mask = np.asarray(inputs["x_padding_masks"], np.float32)
    cov = np.asarray(inputs["coverage_vector"], np.float32)
    Wh = np.ascontiguousarray(np.asarray(inputs["Wh"], np.float32))
    Ws = np.ascontiguousarray(np.asarray(inputs["Ws_w"], np.float32))
    Wsb = np.ascontiguousarray(np.asarray(inputs["Ws_b"], np.float32)[None, :])
    wc = np.ascontiguousarray(np.asarray(inputs["wc"], np.float32)[None, :])
    v = np.asarray(inputs["v"], np.float32)

    sT_full = np.concatenate([h, c], axis=1).T          # (D, B)
    vcols = np.ascontiguousarray(v.reshape(KC, P).T)    # (P, KC)
    ident = np.eye(P, dtype=np.float32)

    nc = _get_nc()
    in_maps = []
    for i in range(NCORES):
        sl = slice(BL * i, BL * (i + 1))
        cov_i = np.ascontiguousarray(cov[sl])
        auxr = np.stack([np.stack([cov_i[b], np.ones(L, np.float32)])
                         for b in range(BL)])           # (BL, 2, L)
        in_maps.append({
            "enc": np.ascontiguousarray(enc[sl]),
            "sT": np.ascontiguousarray(sT_full[:, sl]),
            "auxr": auxr,
            "cov": cov_i,
            "mask": np.ascontiguousarray(mask[sl]),
            "Wh": Wh, "Ws": Ws, "Wsb": Wsb, "vcols": vcols, "wc": wc,
            "ident": ident,
        })
    res = run_bass_kernel_spmd(nc, in_maps, core_ids=list(range(NCORES)))
    ctx = np.concatenate([r["ctx"] for r in res.results], axis=0)
    attn = np.concatenate([r["attn"] for r in res.results], axis=0)
    covn = np.concatenate([r["covn"] for r in res.results], axis=0)
    return (ctx, attn, covn)


# revision 6
# speedup vs baseline: 1.3528x; 1.3528x over previous
"""Trainium2 Bass kernel for pointer-generator attention (nn_Attention_35296041239157).

Math (per batch row b):
    s_t      = concat(h_dec, c_dec)                      # (D,)
    dec_feat = s_t @ Ws_w + Ws_b                         # (D,)
    att      = enc @ Wh + dec_feat + cov[:, None] * wc   # (L, D)
    score    = tanh(att) @ v                             # (L,)
    w        = exp(score) * mask                         # (L,)   (no max-sub: |score| <~ 16)
    attn     = w / sum(w)
    context  = attn @ enc                                # (D,)
    covn     = cov + attn

Distribution: pure data-parallel over batch B=16 -> 2 batches on each of the
8 NeuronCores; the small projection weights are replicated. No collectives.

Device dataflow (per core, per L-quarter of 1024 positions):
  1. SWDGE cast-DMA streams enc (f32 HBM) into bf16 natural tiles [128L, 1024D].
  2. The bf16 tiles bounce through a DRAM scratch tile, then one xbar
     transpose-DMA per D-chunk loads encT [128D, 1024L] - no PE transposes,
     which would otherwise both burn PE time (~300 ns per 128x128) and keep
     the HAM clock gate cold (transpose-mode doesn't count as PE-busy).
  3. The m-loop computes att^T = Wh^T @ encT (+ a K=2 matmul folding the
     dec_feat and coverage rank-1 terms), tanh on ScalarE, then the
     score reduction over D runs on the TensorEngine (lhsT = v column).
  4. exp/mask make unnormalized weights w (f32 rows); small PE transposes
     give w columns for the context matvec against the natural bf16 tiles,
     accumulated unnormalized and divided by sum(w) once at the end.

All heavy matmuls are bf16 (f32r streams at 2 cyc/col on silicon; bf16 at 1)
with f32 PSUM accumulation; softmax/normalization stay f32.
"""

import numpy as np

import concourse.bacc as bacc
import concourse.mybir as mybir
from concourse.tile import TileContext
from concourse.bass_utils import run_bass_kernel_spmd

B, L, D = 16, 4096, 1024
NCORES = 8
BL = B // NCORES          # 2 batches per core
P = 128
LQ = 1024                 # L quarter
NQ = L // LQ              # 4
NJQ = LQ // P             # 8 natural 128-row tiles per quarter
LT = 512                  # score-loop slice (matmul N)
NSTQ = LQ // LT           # 2 slices per quarter
NJ = LT // P              # 4 128-row subtiles per slice
KC = D // P               # 8 contraction / output chunks
NH = D // 512             # 2 halves of D

f32 = mybir.dt.float32
bf16 = mybir.dt.bfloat16
FT = mybir.ActivationFunctionType
AX = mybir.AxisListType
ALU = mybir.AluOpType

NAT_BUFS = 20             # [128, 1024] bf16 natural tiles (16 live per quarter)
ENCT_BUFS = 20            # [128, 1024] bf16 transposed tiles (16 live per quarter)


def build():
    nc = bacc.Bacc("TRN2", target_bir_lowering=False, debug=False)

    enc = nc.dram_tensor("enc", [BL, L, D], f32, kind="ExternalInput")
    sT = nc.dram_tensor("sT", [D, BL], f32, kind="ExternalInput")
    auxr_d = nc.dram_tensor("auxr", [BL, 2, L], f32, kind="ExternalInput")
    cov = nc.dram_tensor("cov", [BL, L], f32, kind="ExternalInput")
    mask = nc.dram_tensor("mask", [BL, L], f32, kind="ExternalInput")
    Wh = nc.dram_tensor("Wh", [D, D], f32, kind="ExternalInput")
    Ws = nc.dram_tensor("Ws", [D, D], f32, kind="ExternalInput")
    Wsb = nc.dram_tensor("Wsb", [1, D], f32, kind="ExternalInput")
    vcols = nc.dram_tensor("vcols", [P, KC], f32, kind="ExternalInput")
    wc = nc.dram_tensor("wc", [1, D], f32, kind="ExternalInput")
    ident_d = nc.dram_tensor("ident", [P, P], f32, kind="ExternalInput")

    attn_d = nc.dram_tensor("attn", [BL, L], f32, kind="ExternalOutput")
    covn_d = nc.dram_tensor("covn", [BL, L], f32, kind="ExternalOutput")
    ctx_d = nc.dram_tensor("ctx", [BL, D], f32, kind="ExternalOutput")

    with TileContext(nc) as tc:
        with tc.tile_pool(name="const", bufs=1) as const:
            wh_sb = const.tile([P, KC, D], bf16)
            vc_sb = const.tile([P, KC], bf16)
            sT_sb = const.tile([P, KC, BL], bf16)
            wsb2 = const.tile([BL, D], f32)
            ident = const.tile([P, P], f32)
            aux_l0 = const.tile([BL, D], bf16)  # row0 = wc, row1 = dec_feat[0]
            aux_l1 = const.tile([BL, D], bf16)
            auxr0 = const.tile([2, L], bf16)    # row0 = cov[0], row1 = ones
            auxr1 = const.tile([2, L], bf16)
            dec_nat = const.tile([BL, D], bf16)
            wboth = const.tile([BL, L], f32)    # w rows; becomes attn in place
            ctx0 = const.tile([1, D], f32)
            ctx1 = const.tile([1, D], f32)
            ctxb = const.tile([BL, D], f32)
            denom = const.tile([BL, 1], f32)
            recip = const.tile([BL, 1], f32)

            # f32 -> bf16 casts ride the SWDGE (gpsimd) DMA path.
            for k in range(KC):
                nc.gpsimd.dma_start(out=wh_sb[:, k, :], in_=Wh[k * P:(k + 1) * P, :])
                nc.gpsimd.dma_start(out=sT_sb[:, k, :], in_=sT[k * P:(k + 1) * P, :])
            nc.gpsimd.dma_start(out=vc_sb[:], in_=vcols[:, :])
            nc.sync.dma_start(out=ident[:], in_=ident_d[:, :])
            for b in range(BL):
                nc.sync.dma_start(out=wsb2[b:b + 1, :], in_=Wsb[0:1, :])
            nc.gpsimd.dma_start(out=aux_l0[0:1, :], in_=wc[0:1, :])
            nc.gpsimd.dma_start(out=aux_l1[0:1, :], in_=wc[0:1, :])
            nc.gpsimd.dma_start(out=auxr0[:], in_=auxr_d[0, :, :])
            nc.gpsimd.dma_start(out=auxr1[:], in_=auxr_d[1, :, :])

            # dec_feat = s_t @ Ws + Ws_b   (natural rows [BL, D])
            with tc.tile_pool(name="wsp", bufs=1) as wsp, \
                 tc.tile_pool(name="ps0", bufs=2, space="PSUM") as ps0:
                ws_sb = wsp.tile([P, KC, D], bf16)
                for k in range(KC):
                    nc.gpsimd.dma_start(out=ws_sb[:, k, :],
                                        in_=Ws[k * P:(k + 1) * P, :])
                for n in range(NH):
                    dp = ps0.tile([BL, 512], f32, name="dec_ps")
                    for k in range(KC):
                        nc.tensor.matmul(dp[:], sT_sb[:, k, :],
                                         ws_sb[:, k, n * 512:(n + 1) * 512],
                                         start=(k == 0), stop=(k == KC - 1))
                    nc.vector.tensor_add(dec_nat[:, n * 512:(n + 1) * 512], dp[:],
                                         wsb2[:, n * 512:(n + 1) * 512])

            # aux lhsT row 1 <- dec_feat rows (cross-partition, via DMA)
            nc.sync.dma_start(out=aux_l0[1:2, :], in_=dec_nat[0:1, :])
            nc.sync.dma_start(out=aux_l1[1:2, :], in_=dec_nat[1:2, :])

            with tc.tile_pool(name="work", bufs=3) as work, \
                 tc.tile_pool(name="dram", bufs=1, space="DRAM") as dpool, \
                 tc.tile_pool(name="psum", bufs=2, space="PSUM") as psum:

                for q in range(NQ):
                    Lq = q * LQ
                    # --- natural bf16 tiles + DRAM scratch bounce -------------
                    nat = {}
                    encb = {}
                    for b in range(BL):
                        eb = dpool.tile([LQ, D], bf16, name="encb", tag="encb",
                                        bufs=4)
                        encb[b] = eb
                        for j in range(NJQ):
                            t = work.tile([P, D], bf16, name="enc_nat",
                                          tag="enc_nat", bufs=NAT_BUFS)
                            nc.gpsimd.dma_start(
                                out=t[:], in_=enc[b, Lq + j * P:Lq + (j + 1) * P, :])
                            nc.sync.dma_start(out=eb[j * P:(j + 1) * P, :], in_=t[:])
                            nat[b, j] = t

                    # --- transposed tiles via xbar DMA ------------------------
                    encT = {}
                    for b in range(BL):
                        for k in range(KC):
                            et = work.tile([P, LQ], bf16, name="encT", tag="encT",
                                           bufs=ENCT_BUFS)
                            nc.sync.dma_start(out=et[:],
                                              in_=encb[b][:, k * P:(k + 1) * P],
                                              transpose=True)
                            encT[b, k] = et

                    for stl in range(NSTQ):
                        L0 = Lq + stl * LT
                        o0 = stl * LT
                        # --- attention scores ---------------------------------
                        for b in range(BL):
                            aux_l = aux_l0 if b == 0 else aux_l1
                            auxr = auxr0 if b == 0 else auxr1
                            sc_ps = psum.tile([1, LT], f32, name="score_ps",
                                              tag="score", bufs=2)
                            pend = None
                            for m in range(KC):
                                ap_ = psum.tile([P, LT], f32, name="att_ps",
                                                tag="mmps", bufs=4)
                                nc.tensor.matmul(ap_[:], aux_l[:, m * P:(m + 1) * P],
                                                 auxr[:, L0:L0 + LT],
                                                 start=True, stop=False)
                                for k in range(KC):
                                    nc.tensor.matmul(ap_[:],
                                                     wh_sb[:, k, m * P:(m + 1) * P],
                                                     encT[b, k][:, o0:o0 + LT],
                                                     start=False, stop=(k == KC - 1))
                                th = work.tile([P, LT], bf16, name="tanhT",
                                               tag="tanhT", bufs=3)
                                nc.scalar.activation(th[:], ap_[:], FT.Tanh)
                                if pend is not None:
                                    nc.tensor.matmul(sc_ps[:],
                                                     vc_sb[:, pend[1]:pend[1] + 1],
                                                     pend[0][:],
                                                     start=(pend[1] == 0), stop=False)
                                pend = (th, m)
                            nc.tensor.matmul(sc_ps[:], vc_sb[:, pend[1]:pend[1] + 1],
                                             pend[0][:], start=False, stop=True)

                            # w = exp(score) * mask  -> row b of wboth
                            msk = work.tile([1, LT], f32, name="msk", tag="msk",
                                            bufs=4)
                            nc.sync.dma_start(out=msk[:],
                                              in_=mask[b:b + 1, L0:L0 + LT])
                            if b == 0:
                                wrow = wboth[0:1, L0:L0 + LT]
                                nc.scalar.activation(wrow, sc_ps[:], FT.Exp)
                                nc.vector.tensor_mul(wrow, wrow, msk[:])
                            else:
                                wtmp = work.tile([1, LT], f32, name="wtmp",
                                                 tag="wtmp", bufs=2)
                                nc.scalar.activation(wtmp[:], sc_ps[:], FT.Exp)
                                nc.vector.tensor_mul(wtmp[:], wtmp[:], msk[:])
                                nc.sync.dma_start(out=wboth[1:2, L0:L0 + LT],
                                                  in_=wtmp[:])

                        # --- w columns (both batches) -------------------------
                        wc_ps = psum.tile([P, BL * NJ], f32, name="wcol_ps",
                                          tag="mmps", bufs=4)
                        for j in range(NJ):
                            nc.tensor.transpose(
                                wc_ps[:, BL * j:BL * (j + 1)],
                                wboth[:, L0 + j * P:L0 + (j + 1) * P],
                                ident[0:BL, 0:BL])
                        wcol = work.tile([P, BL * NJ], bf16, name="wcol",
                                         tag="wcol", bufs=2)
                        nc.scalar.copy(wcol[:], wc_ps[:])

                        # --- context accumulation -----------------------------
                        for b in range(BL):
                            ctx_acc = ctx0 if b == 0 else ctx1
                            for n in range(NH):
                                cp = psum.tile([1, 512], f32, name="ctx_ps",
                                               tag="ctxp", bufs=2)
                                for j in range(NJ):
                                    nc.tensor.matmul(
                                        cp[:], wcol[:, BL * j + b:BL * j + b + 1],
                                        nat[b, stl * NJ + j][:,
                                                             n * 512:(n + 1) * 512],
                                        start=(j == 0), stop=(j == NJ - 1))
                                dst = ctx_acc[0:1, n * 512:(n + 1) * 512]
                                if q == 0 and stl == 0:
                                    nc.vector.tensor_copy(dst, cp[:])
                                else:
                                    nc.vector.tensor_add(dst, dst, cp[:])

                # --- epilogue: softmax, coverage, context ---------------------
                covr = work.tile([BL, L], f32, name="covr", tag="covr", bufs=1)
                nc.sync.dma_start(out=covr[:], in_=cov[:, :])

                nc.vector.tensor_reduce(out=denom[:], in_=wboth[:], axis=AX.X,
                                        op=ALU.add)
                nc.vector.reciprocal(recip[:], denom[:])
                nc.vector.tensor_scalar(wboth[:], wboth[:], recip[:], None, ALU.mult)
                nc.vector.tensor_add(covr[:], covr[:], wboth[:])

                nc.vector.tensor_copy(ctxb[0:1, :], ctx0[:])
                nc.sync.dma_start(out=ctxb[1:2, :], in_=ctx1[0:1, :])
                nc.vector.tensor_scalar(ctxb[:], ctxb[:], recip[:], None, ALU.mult)

                nc.sync.dma_start(out=attn_d[:, :], in_=wboth[:])
                nc.sync.dma_start(out=covn_d[:, :], in_=covr[:])
                nc.sync.dma_start(out=ctx_d[:, :], in_=ctxb[:])

    nc.compile()
    return nc


_NC_CACHE = None


def _get_nc():
    global _NC_CACHE
    if _NC_CACHE is None:
        _NC_CACHE = build()
    return _NC_CACHE


def kernel(**inputs):
    enc = np.ascontiguousarray(np.asarray(inputs["encoder_output"], np.float32))
    h = np.asarray(inputs["h_dec"], np.float32)[0]     # (B, 512)
    c = np.asarray(inputs["c_dec"], np.float32)[0]     # (B, 512)
    mask = np.asarray(inputs["x_padding_masks"], np.float32)
    cov = np.asarray(inputs["coverage_vector"], np.float32)
    Wh = np.ascontiguousarray(np.asarray(inputs["Wh"], np.float32))
    Ws = np.ascontiguousarray(np.asarray(inputs["Ws_w"], np.float32))
    Wsb = np.ascontiguousarray(np.asarray(inputs["Ws_b"], np.float32)[None, :])
    wc = np.ascontiguousarray(np.asarray(inputs["wc"], np.float32)[None, :])
    v = np.asarray(inputs["v"], np.float32)

    sT_full = np.concatenate([h, c], axis=1).T          # (D, B)
    vcols = np.ascontiguousarray(v.reshape(KC, P).T)    # (P, KC)
    ident = np.eye(P, dtype=np.float32)

    nc = _get_nc()
    in_maps = []
    for i in range(NCORES):
        sl = slice(BL * i, BL * (i + 1))
        cov_i = np.ascontiguousarray(cov[sl])
        auxr = np.stack([np.stack([cov_i[b], np.ones(L, np.float32)])
                         for b in range(BL)])           # (BL, 2, L)
        in_maps.append({
            "enc": np.ascontiguousarray(enc[sl]),
            "sT": np.ascontiguousarray(sT_full[:, sl]),
            "auxr": auxr,
            "cov": cov_i,
            "mask": np.ascontiguousarray(mask[sl]),
            "Wh": Wh, "Ws": Ws, "Wsb": Wsb, "vcols": vcols, "wc": wc,
            "ident": ident,
        })
    res = run_bass_kernel_spmd(nc, in_maps, core_ids=list(range(NCORES)))
    ctx = np.concatenate([r["ctx"] for r in res.results], axis=0)
    attn = np.concatenate([r["attn"] for r in res.results], axis=0)
    covn = np.concatenate([r["covn"] for r in res.results], axis=0)
    return (ctx, attn, covn)
